# revision 1
# baseline (speedup 1.0000x reference)
"""Trainium2 Bass kernel for the AR(t) recurrence problem.

Math: the recurrence  x_i = sum_j params[j] * x_{i-1-j} + bias  (i in [t, 2t))
is affine in the seed window, so the whole output is

    out = inputs @ M + c

where M [t, t] and c [t] depend only on params/bias. M's columns obey
    m_{k+1} = shift_down(m_k) + m_k[t-1] * p_rev,   m_0 = p_rev
(p_rev = params reversed), an O(t^2) host-side precompute. The device then
does a single dense matmul, data-parallel over 8 cores (512 rows each):

    per core: out_shard[512, 2048] = inT.T @ M + c
    - inT (input shard, transposed on host, bf16) streams as the stationary
      operand in 128x128 tiles
    - M (bf16) streams as the moving operand in [128, 512] tiles
    - accumulate over the 16 contraction tiles into PSUM [128, 2048] strips
    - add c (fp32, exact) during PSUM->SBUF evacuation, DMA out fp32

bf16 is ample precision here: the data-dependent term is strongly
attenuated by the filter (|M| columns ~1e-2 norm) while c is added in fp32;
measured max elementwise relative error vs the fp32 reference ~1.5e-4.
"""

import numpy as np
import ml_dtypes

B = 4096          # batch rows
T = 2048          # time steps == contraction length
NCORES = 8
BS = B // NCORES  # 512 rows per core
P = 128           # partitions
NJ = T // P       # 16 contraction tiles
KB = 512          # matmul moving free-dim (one PSUM bank of fp32)
NKB = T // KB     # 4 k-blocks
NBT = BS // P     # 4 output row-tiles per core

_cache = {}


def _build_and_compile():
    import concourse.mybir as mybir
    from concourse import bacc
    from concourse.tile import TileContext

    nc = bacc.Bacc(
        "TRN2",
        target_bir_lowering=False,
        debug=False,
        enable_asserts=False,
        num_devices=NCORES,
    )
    inT = nc.dram_tensor("art_inT", [T, BS], mybir.dt.bfloat16, kind="ExternalInput")
    mmat = nc.dram_tensor("art_m", [T, T], mybir.dt.bfloat16, kind="ExternalInput")
    cvec = nc.dram_tensor("art_c", [P, T], mybir.dt.float32, kind="ExternalInput")
    out = nc.dram_tensor("art_out", [BS, T], mybir.dt.float32, kind="ExternalOutput")

    with TileContext(nc) as tc:
        with (
            tc.tile_pool(name="weights", bufs=NJ) as mpool,
            tc.tile_pool(name="acts", bufs=NJ) as ipool,
            tc.tile_pool(name="consts", bufs=1) as cpool,
            tc.tile_pool(name="outs", bufs=2) as opool,
            tc.tile_pool(name="warm", bufs=1) as wpool,
            tc.tile_pool(name="acc", bufs=8, space="PSUM") as pspool,
        ):
            # Scratch operand for the PE warm-up matmuls (HAM clock gate).
            wt = wpool.tile([P, KB], mybir.dt.bfloat16, name="wt")
            nc.vector.memset(wt[:], 0.0)

            c_sb = cpool.tile([P, T], mybir.dt.float32, name="c_sb")

            # Loads alternate between the two HWDGE rings (sync + scalar) so
            # descriptor-generation (~600ns per dma_start) is not the pacing
            # bottleneck; m[n] and in[n] ride opposite rings so the first m
            # tile is not queued behind the first in tile.
            in_tiles, m_tiles = [], []
            for n in range(NJ):
                mq = nc.scalar if n % 2 == 0 else nc.sync
                iq = nc.sync if n % 2 == 0 else nc.scalar
                it = ipool.tile([P, BS], mybir.dt.bfloat16, tag="in", name=f"in_sb{n}")
                iq.dma_start(out=it[:], in_=inT[n * P : (n + 1) * P, :])
                mt = mpool.tile([P, T], mybir.dt.bfloat16, tag="m", name=f"m_sb{n}")
                mq.dma_start(out=mt[:], in_=mmat[n * P : (n + 1) * P, :])
                in_tiles.append(it)
                m_tiles.append(mt)
            # c is first needed by the pass-A evacuation (~43us in); issuing
            # it after the m/in streams keeps it from delaying any m tile.
            nc.sync.dma_start(out=c_sb[:], in_=cvec[:])

            # Two passes of two row-tiles each: 2 PSUM strips of [128, 2048]
            # (4 banks each) per pass fills all 8 banks; the n-loop touches
            # each streamed m tile once per pass so the first pass overlaps
            # the M DMA.
            for half in range(NBT // 2):
                # One 1-bank PSUM tile per (row-tile, k-block) unit: Tile
                # serializes DVE reads vs PE writes at tile granularity, so
                # separate tiles let each bank's evacuation start as soon as
                # its own accumulation group closes.
                ps = [
                    [
                        pspool.tile(
                            [P, KB], mybir.dt.float32, tag="ps", name=f"ps{half}_{bi}_{kb}"
                        )
                        for kb in range(NKB)
                    ]
                    for bi in range(2)
                ]
                if half == 0:
                    # Dummy matmuls on memset data while the first DMAs
                    # land, so the HAM clock gate opens to 8/8 before the
                    # real matmuls. Sized to end just past m0's arrival
                    # (~10.8us) -- more would delay the real stream. Each is
                    # its own start/stop group; the real n==0 matmul
                    # re-clears the bank.
                    for i in range(5):
                        nc.tensor.matmul(ps[0][0][:], wt[:, :P], wt[:])
                def mm(bi, n):
                    bt = half * 2 + bi
                    lhsT = in_tiles[n][:, bt * P : (bt + 1) * P]
                    for kb in range(NKB):
                        nc.tensor.matmul(
                            ps[bi][kb][:],
                            lhsT,
                            m_tiles[n][:, kb * KB : (kb + 1) * KB],
                            start=(n == 0),
                            stop=(n == NJ - 1),
                        )

                # Interleave b-tiles over most of the contraction (matches
                # the M DMA arrival order), but run the last `split`
                # iterations per-b-tile so strip 0's accumulation closes
                # early: its serial DVE evacuation then hides under strip
                # 1's remaining matmuls instead of stalling what follows
                # (pass transition, kernel tail). Pass A keeps split small
                # (m14/m15 DMA arrival is tight); pass B is SBUF-resident.
                split = 4 if half == 0 else 6
                for n in range(NJ - split):
                    for bi in range(2):
                        mm(bi, n)
                for bi in range(2):
                    if half == 1 and bi == 1:
                        # Very last strip: kb-major so each PSUM bank's
                        # accumulation group closes progressively early and
                        # the DVE streams through all 8 tail adds without
                        # idling; only bank 3's add+store trail the last
                        # matmul.
                        bt = half * 2 + bi
                        for kb in range(NKB):
                            for n in range(NJ - split, NJ):
                                nc.tensor.matmul(
                                    ps[bi][kb][:],
                                    in_tiles[n][:, bt * P : (bt + 1) * P],
                                    m_tiles[n][:, kb * KB : (kb + 1) * KB],
                                    start=False,
                                    stop=(n == NJ - 1),
                                )
                    else:
                        for n in range(NJ - split, NJ):
                            mm(bi, n)
                for bi in range(2):
                    bt = half * 2 + bi
                    stq = nc.sync if bi == 0 else nc.scalar
                    ot = opool.tile([P, T], mybir.dt.float32, tag="o", name=f"o_sb{bt}")
                    for kb in range(NKB):
                        ksl = slice(kb * KB, (kb + 1) * KB)
                        nc.vector.tensor_add(
                            out=ot[:, ksl], in0=ps[bi][kb][:], in1=c_sb[:, ksl]
                        )
                        stq.dma_start(
                            out=out[bt * P : (bt + 1) * P, ksl], in_=ot[:, ksl]
                        )

    nc.compile()
    return nc


def _build_M_c(params, bias):
    """M [t, t], c [t] (float64) such that out = inputs @ M + c."""
    t = params.shape[0]
    p_rev = params[::-1].astype(np.float64)
    M = np.empty((t, t), np.float64)
    col = p_rev.copy()
    M[:, 0] = col
    for k in range(1, t):
        last = col[-1]
        shifted = np.empty_like(col)
        shifted[0] = 0.0
        shifted[1:] = col[:-1]
        col = shifted + last * p_rev
        M[:, k] = col
    b = np.float64(bias[0])
    u = np.zeros(t, np.float64)
    c = np.empty(t, np.float64)
    for k in range(t):
        nv = u @ p_rev + b
        c[k] = nv
        u = np.roll(u, -1)
        u[-1] = nv
    return M, c


def _make_in_maps(inputs, params, bias):
    M, c = _build_M_c(params, bias)
    m_bf = M.astype(np.float32).astype(ml_dtypes.bfloat16)
    c128 = np.ascontiguousarray(
        np.broadcast_to(c.astype(np.float32)[None, :], (P, T))
    )
    in_bf = inputs.astype(ml_dtypes.bfloat16)
    in_maps = []
    for s in range(NCORES):
        shard = np.ascontiguousarray(in_bf[s * BS : (s + 1) * BS, :].T)
        in_maps.append({"art_inT": shard, "art_m": m_bf, "art_c": c128})
    return in_maps


def run(inputs, params, bias, **spmd_kwargs):
    """Build in_maps, run the SPMD kernel, return (output, BassKernelResults)."""
    from concourse.bass_utils import run_bass_kernel_spmd

    if "nc" not in _cache:
        _cache["nc"] = _build_and_compile()
    nc = _cache["nc"]

    inputs = np.ascontiguousarray(np.asarray(inputs, dtype=np.float32))
    params = np.asarray(params, dtype=np.float32)
    bias = np.asarray(bias, dtype=np.float32)
    assert inputs.shape == (B, T), inputs.shape
    assert params.shape == (T,), params.shape
    in_maps = _make_in_maps(inputs, params, bias)
    res = run_bass_kernel_spmd(nc, in_maps, core_ids=list(range(NCORES)), **spmd_kwargs)
    out = np.concatenate([r["art_out"] for r in res.results], axis=0)
    return out, res


def kernel(inputs, params, bias):
    out, _ = run(inputs, params, bias)
    return out



# revision 2
# speedup vs baseline: 1.6909x; 1.6909x over previous
"""Trainium2 Bass kernel for the AR(t) recurrence problem.

Math: the recurrence  x_i = sum_j params[j] * x_{i-1-j} + bias  (i in [t, 2t))
is affine in the seed window, so the whole output is

    out = inputs @ M + c

where M [t, t] and c [t] depend only on params/bias (host-precomputed exactly,
O(t^2)). The device does a single dense matmul, data-parallel over 8 cores
(512 rows each), in fp8e4 with the DoubleRow perf mode (2 contraction k-tiles
per PE pass):

    per core: psum[512, 2048] = in8.T @ M8      (fp8e4 x fp8e4 -> fp32 psum)

    - in8: input shard, [jw=128, jt=16, b=512] fp8e4 (scale 1)
    - M8:  M * 2^12,    [jw=128, jt=16, i=2048] fp8e4
    - DoubleRow matmuls: lhsT = in8[:, r:r+2, bt*128:+128]  (K=2x128, M=128)
                         rhs  = M8[:, r:r+2, cb*512:+512]   (N=512)
    - psum evacuated to bf16 (alternating DVE/Act), DMA out bf16
    - host: out = bf16 / 2^12 + c   (c added exactly in fp32/f64)

fp8 is ample precision: the data term inputs@M has magnitude ~0.0155 rms while
|out| ~ 1.8 (dominated by c, added exactly); measured end-to-end max
elementwise relative error vs the fp32 reference ~3.5e-3 (tolerance 2e-2).
"""

import numpy as np
import ml_dtypes

B = 4096          # batch rows
T = 2048          # time steps == contraction length
NCORES = 8
BS = B // NCORES  # 512 rows per core
P = 128           # partitions
NJ = T // P       # 16 contraction tiles
NPAIR = NJ // 2   # 8 DoubleRow contraction pairs
KB = 512          # matmul moving free-dim (one PSUM bank of fp32)
NKB = T // KB     # 4 k-blocks
NBT = BS // P     # 4 output row-tiles per core
MSCALE = 4096.0   # 2^12 scale on M for fp8e4 range

E4 = ml_dtypes.float8_e4m3

_cache = {}


def _build_and_compile():
    import concourse.mybir as mybir
    from concourse import bacc
    from concourse.tile import TileContext

    nc = bacc.Bacc(
        "TRN2",
        target_bir_lowering=False,
        debug=False,
        enable_asserts=False,
        num_devices=NCORES,
    )
    in8 = nc.dram_tensor("art_in8", [P, NJ, BS], mybir.dt.float8e4, kind="ExternalInput")
    m8 = nc.dram_tensor("art_m8", [P, NJ, T], mybir.dt.float8e4, kind="ExternalInput")
    out = nc.dram_tensor("art_out", [BS, T], mybir.dt.bfloat16, kind="ExternalOutput")

    DR = mybir.MatmulPerfMode.DoubleRow

    with TileContext(nc) as tc:
        with (
            tc.tile_pool(name="weights", bufs=NPAIR) as mpool,
            tc.tile_pool(name="acts", bufs=NPAIR) as ipool,
            tc.tile_pool(name="outs", bufs=2) as opool,
            tc.tile_pool(name="warm", bufs=1) as wpool,
            tc.tile_pool(name="acc", bufs=8, space="PSUM") as pspool,
        ):
            # Scratch operand for the PE warm-up matmuls (HAM clock gate).
            wt = wpool.tile([P, 2, KB], mybir.dt.float8e4, name="wt")
            nc.vector.memset(wt[:], 0.0)

            # Stream the contraction pair-chunks: per pair r, the M chunk
            # (512KB) and the in chunk (128KB) ride opposite HWDGE rings so
            # both are ready together; pairs alternate rings to halve the
            # per-ring load.
            in_tiles, m_tiles = [], []
            for r in range(NPAIR):
                mq = nc.scalar if r % 2 == 0 else nc.sync
                iq = nc.sync if r % 2 == 0 else nc.scalar
                it = ipool.tile([P, 2, BS], mybir.dt.float8e4, tag="in", name=f"in_sb{r}")
                iq.dma_start(out=it[:], in_=in8[:, 2 * r : 2 * r + 2, :])
                mt = mpool.tile([P, 2, T], mybir.dt.float8e4, tag="m", name=f"m_sb{r}")
                mq.dma_start(out=mt[:], in_=m8[:, 2 * r : 2 * r + 2, :])
                in_tiles.append(it)
                m_tiles.append(mt)

            # Two passes of two row-tiles each: 8 PSUM banks per pass.
            for half in range(NBT // 2):
                ps = [
                    [
                        pspool.tile(
                            [P, KB], mybir.dt.float32, tag="ps", name=f"ps{half}_{bi}_{kb}"
                        )
                        for kb in range(NKB)
                    ]
                    for bi in range(2)
                ]
                if half == 0:
                    # Dummy matmuls on memset data while the first DMAs land,
                    # so the HAM clock gate opens before the real matmuls.
                    for i in range(5):
                        nc.tensor.matmul(
                            ps[0][0][:], wt[:, :, :P], wt[:], perf_mode=DR
                        )

                def mm(bi, r):
                    bt = half * 2 + bi
                    lhsT = in_tiles[r][:, :, bt * P : (bt + 1) * P]
                    for kb in range(NKB):
                        nc.tensor.matmul(
                            ps[bi][kb][:],
                            lhsT,
                            m_tiles[r][:, :, kb * KB : (kb + 1) * KB],
                            start=(r == 0),
                            stop=(r == NPAIR - 1),
                            perf_mode=DR,
                        )

                # Bulk of the contraction interleaves the two row-tiles (in
                # DMA arrival order); the last pair runs per-(bi, kb) so each
                # PSUM bank's group closes progressively and evacuation
                # streams through.
                for r in range(NPAIR - 1):
                    for bi in range(2):
                        mm(bi, r)
                ots = []
                for bi in range(2):
                    bt = half * 2 + bi
                    ot = opool.tile([P, T], mybir.dt.bfloat16, tag="o", name=f"o_sb{bt}")
                    ots.append(ot)
                    for kb in range(NKB):
                        nc.tensor.matmul(
                            ps[bi][kb][:],
                            in_tiles[NPAIR - 1][:, :, bt * P : (bt + 1) * P],
                            m_tiles[NPAIR - 1][:, :, kb * KB : (kb + 1) * KB],
                            start=False,
                            stop=True,
                            perf_mode=DR,
                        )
                        ksl = slice(kb * KB, (kb + 1) * KB)
                        # Alternate evacuation between DVE and Act so neither
                        # engine paces the kernel.
                        if kb % 2 == 0:
                            nc.vector.tensor_scalar_mul(ot[:, ksl], ps[bi][kb][:], 1.0)
                        else:
                            nc.scalar.copy(ot[:, ksl], ps[bi][kb][:])
                for bi in range(2):
                    bt = half * 2 + bi
                    stq = nc.sync if bi == 0 else nc.scalar
                    for hh in range(2):
                        hsl = slice(hh * (T // 2), (hh + 1) * (T // 2))
                        stq.dma_start(out=out[bt * P : (bt + 1) * P, hsl], in_=ots[bi][:, hsl])

    nc.compile()
    return nc


def _build_M_c(params, bias):
    """M [t, t], c [t] (float64) such that out = inputs @ M + c."""
    t = params.shape[0]
    p_rev = params[::-1].astype(np.float64)
    M = np.empty((t, t), np.float64)
    col = p_rev.copy()
    M[:, 0] = col
    for k in range(1, t):
        last = col[-1]
        shifted = np.empty_like(col)
        shifted[0] = 0.0
        shifted[1:] = col[:-1]
        col = shifted + last * p_rev
        M[:, k] = col
    b = np.float64(bias[0])
    u = np.zeros(t, np.float64)
    c = np.empty(t, np.float64)
    for k in range(t):
        nv = u @ p_rev + b
        c[k] = nv
        u = np.roll(u, -1)
        u[-1] = nv
    return M, c


def _make_in_maps(inputs, params, bias):
    M, c = _build_M_c(params, bias)
    # M8[jw, jt, i] = (M * 2^12)[128*jt + jw, i] in fp8e4
    m8 = np.ascontiguousarray(
        (M * MSCALE).astype(np.float32).astype(E4).reshape(NJ, P, T).transpose(1, 0, 2)
    )
    in8_full = inputs.astype(E4)
    in_maps = []
    for s in range(NCORES):
        shard = in8_full[s * BS : (s + 1) * BS, :]  # [BS, T]
        # in8[jw, jt, b] = shard[b, 128*jt + jw]
        in8 = np.ascontiguousarray(shard.T.reshape(NJ, P, BS).transpose(1, 0, 2))
        in_maps.append({"art_in8": in8, "art_m8": m8})
    return in_maps, c


def run(inputs, params, bias, **spmd_kwargs):
    """Build in_maps, run the SPMD kernel, return (output, BassKernelResults)."""
    from concourse.bass_utils import run_bass_kernel_spmd

    if "nc" not in _cache:
        _cache["nc"] = _build_and_compile()
    nc = _cache["nc"]

    inputs = np.ascontiguousarray(np.asarray(inputs, dtype=np.float32))
    params = np.asarray(params, dtype=np.float32)
    bias = np.asarray(bias, dtype=np.float32)
    assert inputs.shape == (B, T), inputs.shape
    assert params.shape == (T,), params.shape
    in_maps, c = _make_in_maps(inputs, params, bias)
    res = run_bass_kernel_spmd(nc, in_maps, core_ids=list(range(NCORES)), **spmd_kwargs)
    scale = np.float32(1.0 / MSCALE)
    c32 = c.astype(np.float32)
    out = np.concatenate(
        [r["art_out"].astype(np.float32) * scale + c32[None, :] for r in res.results],
        axis=0,
    )
    return out, res


def kernel(inputs, params, bias):
    out, _ = run(inputs, params, bias)
    return out
